# revision 68
# baseline (speedup 1.0000x reference)
"""Trainium2 Bass kernel for nn_Attention (CBAM-style channel+spatial attention).

Computes, for x [4, 32, 64, 64, 64]:
  ca[b, c]       = sigmoid(MLP(concat(mean_dhw(x), max_dhw(x))))
  sa[b, d, h, w] = sigmoid(conv2(relu(conv1(concat(mean_c(x), max_c(x))))))
  attention      = sa * ca;  anti_attention = 1 - attention

Sharded over 8 NeuronCores as (batch, D-half); each core gets a host-padded
40-plane slab (4 halo planes each side) pre-rearranged into the on-chip
layout.  Cross-core traffic is one pair-wise AllGather of 64 stats floats
(only the partner core's half-volume stats are needed for ca).

v3: pipelined schedule.  Keys vs v2: (a) one fat half-chunk DMA per
queue per chunk (128 x 8KB descriptors) so DGE descriptor-gen never
throttles the stream and the scalar queue stays clear for ACT; (b) a
warmup AllGather at t=0 absorbs the CC mesh-setup latency, cutting the
real collective to ~5-10us; (c) the whole ca chain (spatial-stat
pyramids, one-hot PE spatial sums, collective staging, MLP) runs at
high scheduler priority, entirely off the PE queue; (d) the h2
partition shifts for s_conv run as tiny PE matmuls (swap-identity
weights) instead of SBUF-SBUF DMAs, which were queueing behind x in
the DMA engines; (e) outputs are [128, 4096] tiles: 16 output DMAs
total, anti via one fat 4x tensor_scalar each, attn on sync / anti on
scalar HWDGE queues.
"""
import numpy as np
import ml_dtypes

F16 = np.float16
F8 = ml_dtypes.float8_e4m3

B, C, D, H, W = 4, 32, 64, 64, 64
K = 7
NCORES = 8
HALO = 4
DL = 40            # local planes per core (32 own + 2*4 halo)
NCHUNK = 5         # 8-plane chunks
CP = 8             # planes per chunk
PFC = CP * 32      # f-cols per channel per chunk (d_loc*32 + h//2) = 256
HP = H + 6         # padded h extent in s_conv (70)
NVOX = float(D * H * W)
WS = 8.0           # conv1 weight pre-scale (fp8 subnormal safety)
NPAIR = 24         # DoubleRow tap pairs; tap (6,5) runs single

# DoubleRow pairs of conv taps t = kz*7 + ky.  The hw rejects a pair-stride
# of 1 byte, so pair (ky, ky+2) within a kz row (stride 2) and pair the
# ky==5 leftovers across adjacent kz rows (stride 70).
TAP_PAIRS = []
for _kz in range(7):
    TAP_PAIRS += [(_kz * 7 + 0, _kz * 7 + 2), (_kz * 7 + 1, _kz * 7 + 3),
                  (_kz * 7 + 4, _kz * 7 + 6)]
TAP_PAIRS += [((2 * _m) * 7 + 5, (2 * _m + 1) * 7 + 5) for _m in range(3)]
TAP_SINGLE = 6 * 7 + 5
assert len(TAP_PAIRS) == NPAIR

CHUNK_ORDER = [0, 1, 2, 4, 3]   # ca chunks early, halo 4th (conv-g0 gate), 3 last

_CACHE = {}


def _build_nc():
    import concourse.bacc as bacc
    import concourse.mybir as mybir
    from concourse import tile
    from concourse import bass
    from concourse import bass_isa

    f32 = mybir.dt.float32
    bf16 = mybir.dt.float16
    fp8 = mybir.dt.float8e4
    Alu = mybir.AluOpType
    Act = mybir.ActivationFunctionType
    Ax = mybir.AxisListType
    DR = mybir.MatmulPerfMode.DoubleRow

    nc = bacc.Bacc("TRN2", target_bir_lowering=False, debug=False,
                   num_devices=NCORES)

    # ---- external I/O ----
    x_ext = nc.declare_dram_parameter("x", [NCHUNK, 128, 32 * PFC], bf16, isOutput=False)
    convw_ext = nc.declare_dram_parameter("convw", [128, NPAIR * 2 * 2 * 128 + 2 * 128], fp8, isOutput=False)
    ohpb_ext = nc.declare_dram_parameter("ohpb", [128, 256], bf16, isOutput=False)
    id_ext = nc.declare_dram_parameter("ident", [64, 64], f32, isOutput=False)
    idb_ext = nc.declare_dram_parameter("identb", [128, 256], fp8, isOutput=False)
    idbB_ext = nc.declare_dram_parameter("identbB", [128, 128], bf16, isOutput=False)
    c2_ext = nc.declare_dram_parameter("c2w", [128, 128], bf16, isOutput=False)
    fc1w_ext = nc.declare_dram_parameter("fc1w", [128, 64], f32, isOutput=False)
    fc1b_ext = nc.declare_dram_parameter("fc1b", [128, 1], f32, isOutput=False)
    fc2w_ext = nc.declare_dram_parameter("fc2w", [32, 128], f32, isOutput=False)
    fc2b_ext = nc.declare_dram_parameter("fc2b", [32, 1], f32, isOutput=False)
    mask_ext = nc.declare_dram_parameter("masks", [4, 2], f32, isOutput=False)
    swapA_ext = nc.declare_dram_parameter("swapA", [128, 64], fp8, isOutput=False)
    swapB_ext = nc.declare_dram_parameter("swapB", [64, 64], bf16, isOutput=False)
    attn_ext = nc.declare_dram_parameter("attn", [4, 2, 128, 4096], bf16, isOutput=True)
    anti_ext = nc.declare_dram_parameter("anti", [4, 2, 128, 4096], bf16, isOutput=True)

    PAIRS = [[2 * i, 2 * i + 1] for i in range(NCORES // 2)]
    ccw_in = nc.dram_tensor("ccw_in", [1, 4], f32)
    ccw_out = nc.dram_tensor("ccw_out", [2, 4], f32)
    cc_in = nc.dram_tensor("cc_in", [2, 32], f32)
    cc_out = nc.dram_tensor("cc_out", [4, 32], f32)

    with tile.TileContext(nc) as tc:
        with (
            tc.tile_pool(name="consts", bufs=1) as consts,
            tc.tile_pool(name="xpool", bufs=5) as xpool,
            tc.tile_pool(name="sconv", bufs=1) as sconvp,
            tc.tile_pool(name="small", bufs=2) as small,
            tc.tile_pool(name="tree", bufs=1) as treep,
            tc.tile_pool(name="pyr", bufs=1) as pyrp,
            tc.tile_pool(name="relu", bufs=2) as relup,
            tc.tile_pool(name="saw", bufs=2) as sawp,
            tc.tile_pool(name="stat", bufs=1) as statp,
            tc.tile_pool(name="outp", bufs=3) as outp,
            tc.tile_pool(name="pcs", bufs=1, space="PSUM") as pcsp,
            tc.tile_pool(name="psp", bufs=1, space="PSUM") as pspp,
            tc.tile_pool(name="pconv", bufs=2, space="PSUM") as pconvp,
            tc.tile_pool(name="psh", bufs=2, space="PSUM") as pshp,
            tc.tile_pool(name="ptp", bufs=1, space="PSUM") as ptpp,
            tc.tile_pool(name="pmisc", bufs=1, space="PSUM") as pmiscp,
        ):
            # ---- warmup collective: absorbs the CC channel-setup latency so
            # the real AllGather (issued ~35us in) completes quickly ----
            nc.gpsimd.collective_compute(
                "AllGather", mybir.AluOpType.bypass,
                replica_groups=PAIRS,
                ins=[ccw_in[:].opt()], outs=[ccw_out[:].opt()])

            # ---- constants (scalar queue: idle early; keeps sync/gpsimd
            # free for the fat x-chunk DMAs) ----
            ohpb = consts.tile([128, 256], bf16)
            nc.scalar.dma_start(ohpb[:], ohpb_ext[:])
            ident = consts.tile([64, 64], f32)
            nc.scalar.dma_start(ident[:], id_ext[:])
            identb = consts.tile([128, 256], fp8)
            nc.scalar.dma_start(identb[:], idb_ext[:])
            identbB = consts.tile([128, 128], bf16)
            nc.scalar.dma_start(identbB[:], idbB_ext[:])
            c2w = consts.tile([128, 128], bf16)
            nc.scalar.dma_start(c2w[:], c2_ext[:])
            fc1w = consts.tile([128, 64], f32)
            nc.scalar.dma_start(fc1w[:], fc1w_ext[:])
            fc1b = consts.tile([128, 1], f32)
            nc.scalar.dma_start(fc1b[:], fc1b_ext[:])
            fc2w = consts.tile([32, 128], f32)
            nc.scalar.dma_start(fc2w[:], fc2w_ext[:])
            fc2b = consts.tile([32, 1], f32)
            nc.scalar.dma_start(fc2b[:], fc2b_ext[:])
            masks = consts.tile([4, 2], f32)
            nc.scalar.dma_start(masks[:], mask_ext[:])
            swapA = consts.tile([128, 64], fp8)
            nc.scalar.dma_start(swapA[:], swapA_ext[:])
            swapB = consts.tile([64, 64], bf16)
            nc.scalar.dma_start(swapB[:], swapB_ext[:])
            convw = consts.tile([128, NPAIR * 2 * 2 * 128 + 2 * 128], fp8)

            # warm the ACT sigmoid/relu table set off the critical path
            warm = consts.tile([1, 1], f32)
            nc.vector.memset(warm[:], 0.0)
            warm2 = consts.tile([1, 1], f32)
            nc.scalar.activation(warm2[:], warm[:], Act.Sigmoid)

            # persistent tiles
            s_conv = sconvp.tile([128, DL * HP], fp8)       # rows: i*64+w; f: d*70+3+h
            # only the h-pad columns need zeroing; every (plane, 3..66) col
            # is written by the stats stages (halo planes carry host zeros)
            dall = s_conv[:].rearrange("p (d h) -> p d h", d=DL)
            nc.vector.memset(dall[:, :, 0:3], 0.0)
            nc.vector.memset(dall[:, :, 67:70], 0.0)
            sa128 = statp.tile([128, 1024], bf16)           # p=(do%2)*64+h, f=(do//2)*64+w
            ca_rep = statp.tile([128, 32], f32)
            psum_sp = pspp.tile([16, 512], f32)             # pair-wise spatial sums (chunks 0,1)

            sp_parts = [None] * 4                           # per-chunk spatial-max [128,32] f32
            relu_tiles = [[None, None] for _ in range(4)]
            x_tiles = [None] * NCHUNK

            def xdma_all():
                # two half-chunk dma_starts per chunk (one per queue, 128
                # descriptors x 8KB each): minimal DGE serialization AND
                # staggered in-order chunk arrival.  scalar stays free for ACT.
                for k in CHUNK_ORDER:
                    x_k = xpool.tile([128, 32 * PFC], bf16, tag="xk")
                    x_tiles[k] = x_k
                    nc.sync.dma_start(x_k[:, 0:4096], x_ext[k, :, 0:4096])
                    nc.gpsimd.dma_start(x_k[:, 4096:8192], x_ext[k, :, 4096:8192])
                # conv weights last: the ca-critical x chunks go first and
                # convw streams in during conv g0
                nc.sync.dma_start(convw[:, 0:6272], convw_ext[:, 0:6272])
                nc.gpsimd.dma_start(convw[:, 6272:12544], convw_ext[:, 6272:12544])

            def chunk_dparts(k):
                # chunk->s_conv plane mapping: chunks 0-3 = planes 4+8k..12+8k,
                # chunk 4 first half = planes 0..4, second half = planes 36..40
                if k < 4:
                    return [(slice(0, CP), 4 + k * CP)]
                return [(slice(0, 4), 0), (slice(4, 8), 36)]

            def chsum(k, do_spsum=False):
                # channel-sum: identity-matmul accumulation over the 32
                # channels; the spatial sums (ca-critical, high priority)
                # are emitted first so the scheduler runs them at x arrival
                x_k = x_tiles[k]
                if do_spsum:
                    with tc.high_priority():
                        for m in range(16):
                            nc.tensor.matmul(
                                psum_sp[:], ohpb[:, m * 16:(m + 1) * 16],
                                x_k[:, m * 512:(m + 1) * 512],
                                start=(k == 0 and m == 0),
                                stop=(k == 3 and m == 15),
                                skip_group_check=True)
                pcs = pcsp.tile([128, PFC], f32, tag="pcs")
                for m in range(32):
                    nc.tensor.matmul(pcs[:], identbB[:],
                                     x_k[:, m * 256:(m + 1) * 256],
                                     start=(m == 0), stop=(m == 31),
                                     skip_group_check=True)
                # avg half of s_conv (even h) via ACT, odd-h staging via shift
                src_av = pcs[:].rearrange("p (d hh) -> p d hh", d=CP)
                for dsl, d0 in chunk_dparts(k):
                    nds = dsl.stop - dsl.start
                    nc.scalar.activation(
                        dall[0:64, d0:d0 + nds, 3:67:2], src_av[0:64, dsl],
                        Act.Copy, scale=1.0 / 32.0)
                tmp_av = small.tile([128, PFC], fp8, tag="tmpav")
                nc.scalar.activation(tmp_av[64:128, :], pcs[64:128, :], Act.Copy,
                                     scale=1.0 / 32.0)
                # partition shift 64->0 via a tiny PE matmul (no DMA: the DMA
                # engines are saturated with x during this phase)
                psh = pshp.tile([128, PFC], f32, tag="psh")
                nc.tensor.matmul(psh[0:64, :], swapA[64:128, :], tmp_av[64:128, :],
                                 start=True, stop=True, skip_group_check=True)
                return psh

            def chmax(k, psh):
                # channel-max: contiguous halving folds (bf16 2x mode)
                x_k = x_tiles[k]
                t1 = treep.tile([128, 4096], bf16, tag="tr1")
                t2 = treep.tile([128, 2048], bf16, tag="tr2")
                t3 = treep.tile([128, 1024], bf16, tag="tr3")
                t4 = treep.tile([128, 512], bf16, tag="tr4")
                cmx = small.tile([128, PFC], bf16, tag="cmx")
                nc.vector.tensor_max(t1[:], x_k[:, 0:4096], x_k[:, 4096:8192])
                nc.vector.tensor_max(t2[:], t1[:, 0:2048], t1[:, 2048:4096])
                nc.vector.tensor_max(t3[:], t2[:, 0:1024], t2[:, 1024:2048])
                nc.vector.tensor_max(t4[:], t3[:, 0:512], t3[:, 512:1024])
                nc.vector.tensor_max(cmx[:], t4[:, 0:256], t4[:, 256:512])

                # ---- s_conv assembly (odd-h halves) for this chunk ----
                # partition shift 0->64 for the h2=0 max half, again on PE
                nc.tensor.matmul(psh[64:128, :], swapB[:], cmx[0:64, :],
                                 start=True, stop=True, skip_group_check=True)
                psh1v = psh[0:64].rearrange("p (d hh) -> p d hh", d=CP)
                cmv = cmx[64:128].rearrange("p (d hh) -> p d hh", d=CP)
                psh2v = psh[64:128].rearrange("p (d hh) -> p d hh", d=CP)
                for dsl, d0 in chunk_dparts(k):
                    dst = dall[:, d0:d0 + (dsl.stop - dsl.start), :]
                    nc.vector.tensor_copy(dst[0:64, :, 4:68:2], psh1v[:, dsl])
                    nc.vector.tensor_copy(dst[64:128, :, 4:68:2], cmv[:, dsl])
                    nc.vector.tensor_copy(dst[64:128, :, 3:67:2], psh2v[:, dsl])

            def spmax(k):
                # per-channel spatial max: within-channel max pyramid.
                # Chunks 2,3 are ca-critical (the collective waits on them);
                # chunks 0,1 yield to the channel-max trees that gate conv g0.
                x_k = x_tiles[k]
                v0 = x_k[:].rearrange("p (c f) -> p c f", c=32)
                with tc.high_priority():
                    m1 = pyrp.tile([128, 4096], bf16, tag="py1")
                    m1v = m1[:].rearrange("p (c f) -> p c f", c=32)
                    nc.vector.tensor_max(m1v[:], v0[:, :, 0:128], v0[:, :, 128:256])
                    m2 = pyrp.tile([128, 2048], bf16, tag="py2")
                    m2v = m2[:].rearrange("p (c f) -> p c f", c=32)
                    nc.vector.tensor_max(m2v[:], m1v[:, :, 0:64], m1v[:, :, 64:128])
                    m3 = pyrp.tile([128, 1024], bf16, tag="py3")
                    m3v = m3[:].rearrange("p (c f) -> p c f", c=32)
                    nc.vector.tensor_max(m3v[:], m2v[:, :, 0:32], m2v[:, :, 32:64])
                    sp = statp.tile([128, 32], f32, tag=f"sp{k}")
                    nc.vector.tensor_reduce(sp[:], m3v[:], axis=Ax.X, op=Alu.max)
                sp_parts[k] = sp

            def stats_finish():
                from concourse import bass_isa
                ctx = tc.high_priority()
                ctx.__enter__()
                # --- spatial sums (PE psum_sp, all 4 chunks) ---
                junkA = statp.tile([16, 256], f32)
                colA = statp.tile([16, 1], f32)
                nc.scalar.activation(junkA[:], psum_sp[:, 0:256], Act.Copy,
                                     accum_out=colA[:])
                junkB = statp.tile([16, 256], f32)
                colB = statp.tile([16, 1], f32)
                nc.scalar.activation(junkB[:], psum_sp[:, 256:512], Act.Copy,
                                     accum_out=colB[:])
                srow = statp.tile([1, 32], f32)
                nc.gpsimd.dma_start(srow[0:1, 0:32:2], colA[:])
                nc.gpsimd.dma_start(srow[0:1, 1:32:2], colB[:])
                nc.gpsimd.dma_start(cc_in[0:1, :], srow[:])
                # --- spatial max: combine 4 chunk partials ---
                mx01 = statp.tile([128, 32], f32)
                nc.vector.tensor_max(mx01[:], sp_parts[0][:], sp_parts[1][:])
                mx23 = statp.tile([128, 32], f32)
                nc.vector.tensor_max(mx23[:], sp_parts[2][:], sp_parts[3][:])
                mxa = statp.tile([128, 32], f32)
                nc.vector.tensor_max(mxa[:], mx01[:], mx23[:])
                mxr = statp.tile([128, 32], f32)
                nc.gpsimd.partition_all_reduce(mxr[:], mxa[:], 128,
                                               bass_isa.ReduceOp.max)
                nc.gpsimd.dma_start(cc_in[1:2, :], mxr[0:1, :])
                nc.gpsimd.collective_compute(
                    "AllGather", mybir.AluOpType.bypass,
                    replica_groups=PAIRS,
                    ins=[cc_in[:].opt()], outs=[cc_out[:].opt()])
                # gather the pair rows split by kind (sum rows 0,2; max
                # rows 1,3) so the combine stays entirely on the gpsimd queue
                gathS = statp.tile([2, 32], f32)
                nc.gpsimd.dma_start(gathS[:], cc_out[0:4:2, :])
                gathM = statp.tile([2, 32], f32)
                nc.gpsimd.dma_start(gathM[:], cc_out[1:4:2, :])
                ctx.__exit__(None, None, None)
                return gathS, gathM

            def ca_post(gaths):
                from concourse import bass_isa
                gathS, gathM = gaths
                ctx = tc.high_priority()
                ctx.__enter__()
                # pair-combine entirely on the gpsimd queue (no cross-engine
                # hops until fc1)
                tSa = statp.tile([2, 32], f32)
                nc.gpsimd.partition_all_reduce(tSa[:], gathS[:], 2,
                                               bass_isa.ReduceOp.add)
                tMa = statp.tile([2, 32], f32)
                nc.gpsimd.partition_all_reduce(tMa[:], gathM[:], 2,
                                               bass_isa.ReduceOp.max)
                hin = statp.tile([1, 64], f32)
                nc.gpsimd.dma_start(hin[0:1, 0:32], tSa[0:1, :])
                nc.gpsimd.dma_start(hin[0:1, 32:64], tMa[0:1, :])
                # fc1 via broadcast + fused mul-accumulate (all off the PE)
                hinb = statp.tile([128, 64], f32)
                nc.gpsimd.partition_broadcast(hinb[:], hin[:])
                junk1 = statp.tile([128, 64], f32)
                h1 = statp.tile([128, 1], f32)
                nc.vector.scalar_tensor_tensor(junk1[:], fc1w[:], 1.0, hinb[:],
                                               op0=Alu.bypass, op1=Alu.mult,
                                               accum_out=h1[:])
                hrelu = statp.tile([128, 1], f32)
                nc.vector.tensor_scalar(hrelu[:], h1[:], fc1b[:], 0.0,
                                        op0=Alu.add, op1=Alu.max)
                # fc2 on DVE too: ca0[c] = sum_j fc2w[c, j] * hrelu[j]
                hrow = statp.tile([1, 128], f32)
                nc.gpsimd.dma_start(hrow[:], hrelu[:])
                hrelB = statp.tile([32, 128], f32)
                nc.gpsimd.partition_broadcast(hrelB[:], hrow[:])
                junk2 = statp.tile([32, 128], f32)
                ca0 = statp.tile([32, 1], f32)
                nc.vector.scalar_tensor_tensor(junk2[:], fc2w[:], 1.0, hrelB[:],
                                               op0=Alu.bypass, op1=Alu.mult,
                                               accum_out=ca0[:])
                ca_col = statp.tile([32, 1], f32)
                nc.scalar.activation(ca_col[:], ca0[:], Act.Sigmoid, bias=fc2b[:])
                ca_row = statp.tile([1, 32], f32)
                nc.gpsimd.dma_start(ca_row[:], ca_col[:])
                nc.gpsimd.partition_broadcast(ca_rep[:], ca_row[:])
                ctx.__exit__(None, None, None)

            # fp8 DoubleRow conv: tap pairs (2j, 2j+1) share one matmul.
            # convw layout: [p, j(24), ph(2), two(2), col(128)] + tail [p, ph(2), col(128)]
            cwv = convw[:, :NPAIR * 512].rearrange("p (j ph two c) -> p j ph two c",
                                                   j=NPAIR, ph=2, two=2)
            cwtail = convw[:, NPAIR * 512:].rearrange("p (t c) -> p t c", c=128)
            sflat = s_conv[:]

            def conv_rhs(g, j):
                t0, t1 = TAP_PAIRS[j]
                kz, ky = t0 // 7, t0 % 7
                delta = (t1 // 7 - kz) * HP + (t1 % 7 - ky)
                off = (8 * g + 1 + kz) * HP + ky
                return bass.AP(tensor=sflat.tensor,
                               offset=sflat.offset + off,
                               ap=[list(sflat.ap[0]), [delta, 2], [HP, 8], [1, 64]])

            def conv_group(g):
                # outputs own planes d_own in [8g, 8g+8) = local d in [8g+4, 8g+12)
                pc_a = pconvp.tile([128, 512], f32, tag="pconv")
                pc_b = pconvp.tile([128, 512], f32, tag="pconv")
                pc = [pc_a, pc_b]
                for j in range(NPAIR):
                    rhs = conv_rhs(g, j)
                    for ph in range(2):
                        nc.tensor.matmul(pc[ph][:], cwv[:, j, ph], rhs,
                                         start=(j == 0), stop=False,
                                         perf_mode=DR, skip_group_check=True)
                # tail tap (kz=6, ky=5), plain fp8 matmul
                toff = (8 * g + 7) * HP + 5
                trhs = bass.AP(tensor=sflat.tensor, offset=sflat.offset + toff,
                               ap=[list(sflat.ap[0]), [HP, 8], [1, 64]])
                for ph in range(2):
                    nc.tensor.matmul(pc[ph][:], cwtail[:, ph], trhs,
                                     start=False, stop=True, skip_group_check=True)
                # relu -> sbuf (descale the fp8 weight pre-scale)
                for ph in range(2):
                    r = relup.tile([128, 512], bf16, tag="relu")
                    nc.scalar.activation(r[:], pc[ph][:], Act.Relu, scale=1.0 / WS)
                    relu_tiles[g][ph] = r
                # conv2 (1x1x1, 4 -> 1) and sigmoid
                psa = pmiscp.tile([64, 512], f32, tag="m")
                nc.tensor.matmul(psa[:], c2w[:, 0:64], relu_tiles[g][0][:],
                                 start=True, stop=False, skip_group_check=True)
                nc.tensor.matmul(psa[:], c2w[:, 64:128], relu_tiles[g][1][:],
                                 start=False, stop=True, skip_group_check=True)
                sa_w = sawp.tile([64, 512], f32, tag="saw")
                nc.scalar.activation(sa_w[:], psa[:], Act.Copy)
                # transpose [64,128] blocks -> sa128, sigmoid fused in the copy
                for b4 in range(4):
                    pt = ptpp.tile([128, 64], f32, tag="ptp")
                    nc.tensor.transpose(pt[:], sa_w[:, b4 * 128:(b4 + 1) * 128],
                                        ident[:])
                    col = (4 * g + b4) * 64
                    nc.scalar.activation(sa128[:, col:col + 64], pt[:], Act.Sigmoid)

            def output_quarter(g):
                # outputs for d_own in [8g, 8g+8): sa128 cols [g*256, (g+1)*256)
                # big [128, 4096] tiles (4 channel-groups) -> 1 DMA + 1 fat
                # anti op per half; attn DMA on sync, anti DMA on scalar
                sl_sa = slice(g * 256, (g + 1) * 256)
                for half in range(2):
                    abuf = outp.tile([128, 4096], bf16, tag="abuf")
                    bbuf = outp.tile([128, 4096], bf16, tag="bbuf")
                    for c16 in range(16):
                        c = half * 16 + c16
                        nc.vector.tensor_scalar_mul(
                            abuf[:, c16 * 256:(c16 + 1) * 256], sa128[:, sl_sa],
                            ca_rep[:, c:c + 1])
                    nc.vector.tensor_scalar(bbuf[:], abuf[:], -1.0, 1.0,
                                            op0=Alu.mult, op1=Alu.add)
                    nc.sync.dma_start(attn_ext[g, half], abuf[:])
                    nc.scalar.dma_start(anti_ext[g, half], bbuf[:])

            # ---- schedule ----
            # PE order: stats matmuls of chunks {0,1,4} -> conv g0 ->
            # chunk 2 -> chunk 3 (all ca-critical stats before conv g1) ->
            # conv g1..g3.  DVE order: spatial-max pyramid before the
            # channel-max tree per chunk; chunk-3 tree after the collective
            # is issued.
            xdma_all()
            for k in (0, 1):
                psh = chsum(k, do_spsum=True)
                spmax(k)
                chmax(k, psh)
            psh_2 = chsum(2, do_spsum=True)
            spmax(2)
            chmax(2, psh_2)
            psh_4 = chsum(4)
            chmax(4, psh_4)
            psh_3 = chsum(3, do_spsum=True)
            spmax(3)
            gath = stats_finish()
            conv_group(0)
            chmax(3, psh_3)
            ca_post(gath)
            conv_group(1)
            conv_group(2)
            conv_group(3)
            output_quarter(0)
            output_quarter(1)
            output_quarter(2)
            output_quarter(3)

    nc.compile()
    return nc


def _host_inputs(x, fc1_w, fc1_b, fc2_w, fc2_b, conv1_w, conv2_w):
    """Build the per-core input maps (all host-side numpy)."""
    x = np.asarray(x, dtype=np.float32)
    # conv1 Toeplitz lhsT blocks: T[t2][(i,w_in), (o2,w_out)]
    w1 = np.asarray(conv1_w, dtype=np.float32)  # [4, 2, 7, 7, 7]
    T = np.zeros((98, 128, 128), np.float32)
    for kz in range(7):
        for ky in range(7):
            t = kz * 7 + ky
            for pair in range(2):
                t2 = t * 2 + pair
                for o2 in range(2):
                    oc = pair * 2 + o2
                    for i in range(2):
                        for dk in range(7):
                            off = dk - 3  # w_in = w_out + off
                            wv = w1[oc, i, kz, ky, dk]
                            if off >= 0:
                                wo = np.arange(0, 64 - off)
                            else:
                                wo = np.arange(-off, 64)
                            T[t2, i * 64 + wo + off, o2 * 64 + wo] = wv
    T *= WS
    # pack DoubleRow pairs: [row, j, ph, two, col]; tail taps 48 at the end
    cw8 = np.zeros((128, NPAIR * 2 * 2 * 128 + 2 * 128), np.float32)
    cwv = cw8[:, :NPAIR * 2 * 2 * 128].reshape(128, NPAIR, 2, 2, 128)
    for j in range(NPAIR):
        for ph in range(2):
            for two in range(2):
                cwv[:, j, ph, two, :] = T[TAP_PAIRS[j][two] * 2 + ph]
    for ph in range(2):
        cw8[:, NPAIR * 512 + ph * 128:NPAIR * 512 + (ph + 1) * 128] = T[TAP_SINGLE * 2 + ph]
    convw8 = cw8.astype(F8)

    # pair one-hot weights for the PE spatial-sum matmuls: matmul m covers
    # channels (2m, 2m+1); psum row m gets their partition sums by f-half
    ohpb = np.zeros((128, 256), F16)
    for m in range(16):
        ohpb[:, m * 16 + m] = 1.0
    ident = np.eye(64, dtype=np.float32)
    # two identity blocks for the DoubleRow channel-sum + a plain bf16 one
    identb = np.concatenate([np.eye(128), np.eye(128)], axis=1).astype(F8)
    identbB = np.eye(128, dtype=np.float32).astype(F16)
    swapA = np.zeros((128, 64), np.float32)
    swapA[64:128] = np.eye(64)
    swapA = swapA.astype(F8)
    swapB = np.eye(64, dtype=np.float32).astype(F16)

    c2v = np.asarray(conv2_w, dtype=np.float32).reshape(4)
    c2 = np.zeros((128, 128), np.float32)
    for pair in range(2):
        for o2 in range(2):
            w = np.arange(64)
            c2[o2 * 64 + w, pair * 64 + w] = c2v[pair * 2 + o2]
    c2 = c2.astype(F16)

    fc1_w = np.asarray(fc1_w, np.float32)           # [128, 64]
    fc1s = fc1_w.copy()
    fc1s[:, 0:32] *= 1.0 / NVOX
    fc1bv = np.asarray(fc1_b, np.float32).reshape(128, 1)
    fc2v = np.ascontiguousarray(np.asarray(fc2_w, np.float32))  # [32, 128]
    masks = np.zeros((4, 2), np.float32)
    masks[0, 0] = masks[2, 0] = 1.0
    masks[1, 1] = masks[3, 1] = 1.0
    fc2bv = np.asarray(fc2_b, np.float32).reshape(32, 1)

    in_maps = []
    for r in range(NCORES):
        b, dhalf = r // 2, r % 2
        xp = np.zeros((C, DL, H, W), np.float32)
        if dhalf == 0:
            xp[:, 4:40] = x[b, :, 0:36]
        else:
            xp[:, 0:36] = x[b, :, 28:64]
        # chunk remap: chunks 0-3 carry own planes 4..35, chunk 4 the halos
        xp = xp[:, list(range(4, 36)) + list(range(0, 4)) + list(range(36, 40))]
        # [c, k, dl, hh, h2, w] -> [k, h2, w, c, dl, hh] -> [5, 128, 8192]
        xr = xp.reshape(C, NCHUNK, CP, 32, 2, W).transpose(1, 4, 5, 0, 2, 3)
        xhost = np.ascontiguousarray(xr.reshape(NCHUNK, 128, 32 * PFC)).astype(F16)

        in_maps.append({
            "x": xhost, "convw": convw8, "ohpb": ohpb, "ident": ident, "identb": identb, "identbB": identbB, "c2w": c2,
            "fc1w": fc1s, "fc1b": fc1bv, "fc2w": fc2v, "fc2b": fc2bv,
            "masks": masks, "swapA": swapA, "swapB": swapB,
        })
    return in_maps


def _decode_out(arr):
    """[4, 2, 128, 4096] -> [C, 32, H, W] (own planes)."""
    a = np.asarray(arr, dtype=np.float32)
    a = a.reshape(4, 2, 2, 64, 16, 4, 64)           # g, half, d2, h, c16, b4, w
    a = a.transpose(1, 4, 0, 5, 2, 3, 6)            # half, c16, g, b4, d2, h, w
    return a.reshape(C, 32, H, W)


def _install_ntff_shim():
    """The agent image's antenv lacks axon_hooks; recreate it so
    run_bass_kernel_spmd(trace=True) can NTFF-profile via libaxon."""
    import sys, types, contextlib, ctypes
    try:
        import antenv.axon_hooks  # noqa
        return
    except ImportError:
        pass
    so_path = "/opt/axon/libaxon_pjrt.so"
    lib = ctypes.CDLL(so_path)
    if not hasattr(lib, "axon_start_nrt_profile"):
        return
    lib.axon_start_nrt_profile.argtypes = [ctypes.POINTER(ctypes.c_int64),
                                           ctypes.c_size_t]
    lib.axon_start_nrt_profile.restype = ctypes.c_int64
    lib.axon_stop_nrt_profile.argtypes = [ctypes.c_char_p]
    lib.axon_stop_nrt_profile.restype = ctypes.c_int64

    @contextlib.contextmanager
    def _hook(output_dir, device_ids):
        import jax
        jax.devices()
        if device_ids:
            ids = (ctypes.c_int64 * len(device_ids))(*device_ids)
            rc = lib.axon_start_nrt_profile(ids, len(device_ids))
        else:
            rc = lib.axon_start_nrt_profile(None, 0)
        if rc != 0:
            raise RuntimeError(f"axon_start_nrt_profile rc={rc}")
        try:
            yield
        finally:
            n = lib.axon_stop_nrt_profile(str(output_dir).encode())
            print(f"profile: {n} file(s) written to {output_dir}")

    mod = types.ModuleType("antenv.axon_hooks")
    _state = {"hook": _hook}
    mod.get_axon_ntff_profile_hook = lambda: _state["hook"]
    mod.set_axon_ntff_profile_hook = lambda h: _state.__setitem__("hook", h)
    sys.modules["antenv.axon_hooks"] = mod


def kernel(x, fc1_w, fc1_b, fc2_w, fc2_b, conv1_w, conv2_w, _want_time=False):
    from concourse.bass_utils import run_bass_kernel_spmd
    if _want_time:
        _install_ntff_shim()

    if "nc" not in _CACHE:
        _CACHE["nc"] = _build_nc()
    nc = _CACHE["nc"]

    in_maps = _host_inputs(x, fc1_w, fc1_b, fc2_w, fc2_b, conv1_w, conv2_w)
    res = run_bass_kernel_spmd(nc, in_maps, core_ids=list(range(NCORES)),
                               trace=bool(_want_time))
    attention = np.empty((B, C, D, H, W), np.float32)
    anti = np.empty((B, C, D, H, W), np.float32)
    for r in range(NCORES):
        b, dhalf = r // 2, r % 2
        d0 = dhalf * 32
        attention[b, :, d0:d0 + 32] = _decode_out(res.results[r]["attn"])
        anti[b, :, d0:d0 + 32] = _decode_out(res.results[r]["anti"])
    if _want_time:
        return (attention, anti), res.exec_time_ns
    return attention, anti


# revision 69
# speedup vs baseline: 1.0142x; 1.0142x over previous
"""Trainium2 Bass kernel for nn_Attention (CBAM-style channel+spatial attention).

Computes, for x [4, 32, 64, 64, 64]:
  ca[b, c]       = sigmoid(MLP(concat(mean_dhw(x), max_dhw(x))))
  sa[b, d, h, w] = sigmoid(conv2(relu(conv1(concat(mean_c(x), max_c(x))))))
  attention      = sa * ca;  anti_attention = 1 - attention

Sharded over 8 NeuronCores as (batch, D-half); each core gets a host-padded
40-plane slab (4 halo planes each side) pre-rearranged into the on-chip
layout.  Cross-core traffic is one pair-wise AllGather of 64 stats floats
(only the partner core's half-volume stats are needed for ca).

v3: pipelined schedule.  Keys vs v2: (a) one fat half-chunk DMA per
queue per chunk (128 x 8KB descriptors) so DGE descriptor-gen never
throttles the stream and the scalar queue stays clear for ACT; (b) a
warmup AllGather at t=0 absorbs the CC mesh-setup latency, cutting the
real collective to ~5-10us; (c) the whole ca chain (spatial-stat
pyramids, one-hot PE spatial sums, collective staging, MLP) runs at
high scheduler priority, entirely off the PE queue; (d) the h2
partition shifts for s_conv run as tiny PE matmuls (swap-identity
weights) instead of SBUF-SBUF DMAs, which were queueing behind x in
the DMA engines; (e) outputs are [128, 4096] tiles: 16 output DMAs
total, anti via one fat 4x tensor_scalar each, attn on sync / anti on
scalar HWDGE queues.
"""
import numpy as np
import ml_dtypes

F16 = np.float16
F8 = ml_dtypes.float8_e4m3

B, C, D, H, W = 4, 32, 64, 64, 64
K = 7
NCORES = 8
HALO = 4
DL = 40            # local planes per core (32 own + 2*4 halo)
NCHUNK = 5         # 8-plane chunks
CP = 8             # planes per chunk
PFC = CP * 32      # f-cols per channel per chunk (d_loc*32 + h//2) = 256
HP = H + 6         # padded h extent in s_conv (70)
NVOX = float(D * H * W)
WS = 8.0           # conv1 weight pre-scale (fp8 subnormal safety)
NPAIR = 24         # DoubleRow tap pairs; tap (6,5) runs single

# DoubleRow pairs of conv taps t = kz*7 + ky.  The hw rejects a pair-stride
# of 1 byte, so pair (ky, ky+2) within a kz row (stride 2) and pair the
# ky==5 leftovers across adjacent kz rows (stride 70).
TAP_PAIRS = []
for _kz in range(7):
    TAP_PAIRS += [(_kz * 7 + 0, _kz * 7 + 2), (_kz * 7 + 1, _kz * 7 + 3),
                  (_kz * 7 + 4, _kz * 7 + 6)]
TAP_PAIRS += [((2 * _m) * 7 + 5, (2 * _m + 1) * 7 + 5) for _m in range(3)]
TAP_SINGLE = 6 * 7 + 5
assert len(TAP_PAIRS) == NPAIR

CHUNK_ORDER = [0, 1, 2, 4, 3]   # ca chunks early, halo 4th (conv-g0 gate), 3 last

_CACHE = {}


def _build_nc():
    import concourse.bacc as bacc
    import concourse.mybir as mybir
    from concourse import tile
    from concourse import bass
    from concourse import bass_isa

    f32 = mybir.dt.float32
    bf16 = mybir.dt.float16
    fp8 = mybir.dt.float8e4
    Alu = mybir.AluOpType
    Act = mybir.ActivationFunctionType
    Ax = mybir.AxisListType
    DR = mybir.MatmulPerfMode.DoubleRow

    nc = bacc.Bacc("TRN2", target_bir_lowering=False, debug=False,
                   num_devices=NCORES)

    # ---- external I/O ----
    x_ext = nc.declare_dram_parameter("x", [NCHUNK, 128, 32 * PFC], bf16, isOutput=False)
    convw_ext = nc.declare_dram_parameter("convw", [128, NPAIR * 2 * 2 * 128 + 2 * 128], fp8, isOutput=False)
    ohpb_ext = nc.declare_dram_parameter("ohpb", [128, 256], bf16, isOutput=False)
    id_ext = nc.declare_dram_parameter("ident", [64, 64], f32, isOutput=False)
    idb_ext = nc.declare_dram_parameter("identb", [128, 256], fp8, isOutput=False)
    idbB_ext = nc.declare_dram_parameter("identbB", [128, 128], bf16, isOutput=False)
    c2_ext = nc.declare_dram_parameter("c2w", [128, 128], bf16, isOutput=False)
    fc1w_ext = nc.declare_dram_parameter("fc1w", [128, 64], f32, isOutput=False)
    fc1b_ext = nc.declare_dram_parameter("fc1b", [128, 1], f32, isOutput=False)
    fc2w_ext = nc.declare_dram_parameter("fc2w", [32, 128], f32, isOutput=False)
    fc2b_ext = nc.declare_dram_parameter("fc2b", [32, 1], f32, isOutput=False)
    mask_ext = nc.declare_dram_parameter("masks", [4, 2], f32, isOutput=False)
    swapA_ext = nc.declare_dram_parameter("swapA", [128, 64], fp8, isOutput=False)
    swapB_ext = nc.declare_dram_parameter("swapB", [64, 64], bf16, isOutput=False)
    attn_ext = nc.declare_dram_parameter("attn", [4, 2, 128, 4096], bf16, isOutput=True)
    anti_ext = nc.declare_dram_parameter("anti", [4, 2, 128, 4096], bf16, isOutput=True)

    PAIRS = [[2 * i, 2 * i + 1] for i in range(NCORES // 2)]
    ccw_in = nc.dram_tensor("ccw_in", [1, 4], f32)
    ccw_out = nc.dram_tensor("ccw_out", [2, 4], f32)
    cc_in = nc.dram_tensor("cc_in", [2, 32], f32)
    cc_out = nc.dram_tensor("cc_out", [4, 32], f32)

    with tile.TileContext(nc) as tc:
        with (
            tc.tile_pool(name="consts", bufs=1) as consts,
            tc.tile_pool(name="xpool", bufs=5) as xpool,
            tc.tile_pool(name="sconv", bufs=1) as sconvp,
            tc.tile_pool(name="small", bufs=2) as small,
            tc.tile_pool(name="tree", bufs=1) as treep,
            tc.tile_pool(name="pyr", bufs=1) as pyrp,
            tc.tile_pool(name="relu", bufs=3) as relup,
            tc.tile_pool(name="saw", bufs=2) as sawp,
            tc.tile_pool(name="stat", bufs=1) as statp,
            tc.tile_pool(name="outp", bufs=4) as outp,
            tc.tile_pool(name="pcs", bufs=1, space="PSUM") as pcsp,
            tc.tile_pool(name="psp", bufs=1, space="PSUM") as pspp,
            tc.tile_pool(name="pconv", bufs=2, space="PSUM") as pconvp,
            tc.tile_pool(name="psh", bufs=2, space="PSUM") as pshp,
            tc.tile_pool(name="ptp", bufs=1, space="PSUM") as ptpp,
            tc.tile_pool(name="pmisc", bufs=1, space="PSUM") as pmiscp,
        ):
            # ---- warmup collective: absorbs the CC channel-setup latency so
            # the real AllGather (issued ~35us in) completes quickly ----
            nc.gpsimd.collective_compute(
                "AllGather", mybir.AluOpType.bypass,
                replica_groups=PAIRS,
                ins=[ccw_in[:].opt()], outs=[ccw_out[:].opt()])

            # ---- constants (scalar queue: idle early; keeps sync/gpsimd
            # free for the fat x-chunk DMAs) ----
            ohpb = consts.tile([128, 256], bf16)
            nc.scalar.dma_start(ohpb[:], ohpb_ext[:])
            ident = consts.tile([64, 64], f32)
            nc.scalar.dma_start(ident[:], id_ext[:])
            identb = consts.tile([128, 256], fp8)
            nc.scalar.dma_start(identb[:], idb_ext[:])
            identbB = consts.tile([128, 128], bf16)
            nc.scalar.dma_start(identbB[:], idbB_ext[:])
            c2w = consts.tile([128, 128], bf16)
            nc.scalar.dma_start(c2w[:], c2_ext[:])
            fc1w = consts.tile([128, 64], f32)
            nc.scalar.dma_start(fc1w[:], fc1w_ext[:])
            fc1b = consts.tile([128, 1], f32)
            nc.scalar.dma_start(fc1b[:], fc1b_ext[:])
            fc2w = consts.tile([32, 128], f32)
            nc.scalar.dma_start(fc2w[:], fc2w_ext[:])
            fc2b = consts.tile([32, 1], f32)
            nc.scalar.dma_start(fc2b[:], fc2b_ext[:])
            masks = consts.tile([4, 2], f32)
            nc.scalar.dma_start(masks[:], mask_ext[:])
            swapA = consts.tile([128, 64], fp8)
            nc.scalar.dma_start(swapA[:], swapA_ext[:])
            swapB = consts.tile([64, 64], bf16)
            nc.scalar.dma_start(swapB[:], swapB_ext[:])
            convw = consts.tile([128, NPAIR * 2 * 2 * 128 + 2 * 128], fp8)

            # warm the ACT sigmoid/relu table set off the critical path
            warm = consts.tile([1, 1], f32)
            nc.vector.memset(warm[:], 0.0)
            warm2 = consts.tile([1, 1], f32)
            nc.scalar.activation(warm2[:], warm[:], Act.Sigmoid)

            # persistent tiles
            s_conv = sconvp.tile([128, DL * HP], fp8)       # rows: i*64+w; f: d*70+3+h
            # only the h-pad columns need zeroing; every (plane, 3..66) col
            # is written by the stats stages (halo planes carry host zeros)
            dall = s_conv[:].rearrange("p (d h) -> p d h", d=DL)
            nc.vector.memset(dall[:, :, 0:3], 0.0)
            nc.vector.memset(dall[:, :, 67:70], 0.0)
            sa128 = statp.tile([128, 1024], bf16)           # p=(do%2)*64+h, f=(do//2)*64+w
            ca_rep = statp.tile([128, 32], f32)
            psum_sp = pspp.tile([16, 512], f32)             # pair-wise spatial sums (chunks 0,1)

            sp_parts = [None] * 4                           # per-chunk spatial-max [128,32] f32
            relu_tiles = [[None, None] for _ in range(4)]
            x_tiles = [None] * NCHUNK

            def xdma_all():
                # two half-chunk dma_starts per chunk (one per queue, 128
                # descriptors x 8KB each): minimal DGE serialization AND
                # staggered in-order chunk arrival.  scalar stays free for ACT.
                for k in CHUNK_ORDER:
                    x_k = xpool.tile([128, 32 * PFC], bf16, tag="xk")
                    x_tiles[k] = x_k
                    nc.sync.dma_start(x_k[:, 0:4096], x_ext[k, :, 0:4096])
                    nc.gpsimd.dma_start(x_k[:, 4096:8192], x_ext[k, :, 4096:8192])
                # conv weights last: the ca-critical x chunks go first and
                # convw streams in during conv g0
                nc.sync.dma_start(convw[:, 0:6272], convw_ext[:, 0:6272])
                nc.gpsimd.dma_start(convw[:, 6272:12544], convw_ext[:, 6272:12544])

            def chunk_dparts(k):
                # chunk->s_conv plane mapping: chunks 0-3 = planes 4+8k..12+8k,
                # chunk 4 first half = planes 0..4, second half = planes 36..40
                if k < 4:
                    return [(slice(0, CP), 4 + k * CP)]
                return [(slice(0, 4), 0), (slice(4, 8), 36)]

            def chsum(k, do_spsum=False):
                # channel-sum: identity-matmul accumulation over the 32
                # channels; the spatial sums (ca-critical, high priority)
                # are emitted first so the scheduler runs them at x arrival
                x_k = x_tiles[k]
                if do_spsum:
                    with tc.high_priority():
                        for m in range(16):
                            nc.tensor.matmul(
                                psum_sp[:], ohpb[:, m * 16:(m + 1) * 16],
                                x_k[:, m * 512:(m + 1) * 512],
                                start=(k == 0 and m == 0),
                                stop=(k == 3 and m == 15),
                                skip_group_check=True)
                pcs = pcsp.tile([128, PFC], f32, tag="pcs")
                for m in range(32):
                    nc.tensor.matmul(pcs[:], identbB[:],
                                     x_k[:, m * 256:(m + 1) * 256],
                                     start=(m == 0), stop=(m == 31),
                                     skip_group_check=True)
                # avg half of s_conv (even h) via ACT, odd-h staging via shift
                src_av = pcs[:].rearrange("p (d hh) -> p d hh", d=CP)
                for dsl, d0 in chunk_dparts(k):
                    nds = dsl.stop - dsl.start
                    nc.scalar.activation(
                        dall[0:64, d0:d0 + nds, 3:67:2], src_av[0:64, dsl],
                        Act.Copy, scale=1.0 / 32.0)
                tmp_av = small.tile([128, PFC], fp8, tag="tmpav")
                nc.scalar.activation(tmp_av[64:128, :], pcs[64:128, :], Act.Copy,
                                     scale=1.0 / 32.0)
                # partition shift 64->0 via a tiny PE matmul (no DMA: the DMA
                # engines are saturated with x during this phase)
                psh = pshp.tile([128, PFC], f32, tag="psh")
                nc.tensor.matmul(psh[0:64, :], swapA[64:128, :], tmp_av[64:128, :],
                                 start=True, stop=True, skip_group_check=True)
                return psh

            def chmax(k, psh):
                # channel-max: contiguous halving folds (bf16 2x mode)
                x_k = x_tiles[k]
                t1 = treep.tile([128, 4096], bf16, tag="tr1")
                t2 = treep.tile([128, 2048], bf16, tag="tr2")
                t3 = treep.tile([128, 1024], bf16, tag="tr3")
                t4 = treep.tile([128, 512], bf16, tag="tr4")
                cmx = small.tile([128, PFC], bf16, tag="cmx")
                nc.vector.tensor_max(t1[:], x_k[:, 0:4096], x_k[:, 4096:8192])
                nc.vector.tensor_max(t2[:], t1[:, 0:2048], t1[:, 2048:4096])
                nc.vector.tensor_max(t3[:], t2[:, 0:1024], t2[:, 1024:2048])
                nc.vector.tensor_max(t4[:], t3[:, 0:512], t3[:, 512:1024])
                nc.vector.tensor_max(cmx[:], t4[:, 0:256], t4[:, 256:512])

                # ---- s_conv assembly (odd-h halves) for this chunk ----
                # partition shift 0->64 for the h2=0 max half, again on PE
                nc.tensor.matmul(psh[64:128, :], swapB[:], cmx[0:64, :],
                                 start=True, stop=True, skip_group_check=True)
                psh1v = psh[0:64].rearrange("p (d hh) -> p d hh", d=CP)
                cmv = cmx[64:128].rearrange("p (d hh) -> p d hh", d=CP)
                psh2v = psh[64:128].rearrange("p (d hh) -> p d hh", d=CP)
                for dsl, d0 in chunk_dparts(k):
                    dst = dall[:, d0:d0 + (dsl.stop - dsl.start), :]
                    nc.vector.tensor_copy(dst[0:64, :, 4:68:2], psh1v[:, dsl])
                    nc.vector.tensor_copy(dst[64:128, :, 4:68:2], cmv[:, dsl])
                    nc.vector.tensor_copy(dst[64:128, :, 3:67:2], psh2v[:, dsl])

            def spmax(k):
                # per-channel spatial max: within-channel max pyramid.
                # Chunks 2,3 are ca-critical (the collective waits on them);
                # chunks 0,1 yield to the channel-max trees that gate conv g0.
                x_k = x_tiles[k]
                v0 = x_k[:].rearrange("p (c f) -> p c f", c=32)
                with tc.high_priority():
                    m1 = pyrp.tile([128, 4096], bf16, tag="py1")
                    m1v = m1[:].rearrange("p (c f) -> p c f", c=32)
                    nc.vector.tensor_max(m1v[:], v0[:, :, 0:128], v0[:, :, 128:256])
                    m2 = pyrp.tile([128, 2048], bf16, tag="py2")
                    m2v = m2[:].rearrange("p (c f) -> p c f", c=32)
                    nc.vector.tensor_max(m2v[:], m1v[:, :, 0:64], m1v[:, :, 64:128])
                    m3 = pyrp.tile([128, 1024], bf16, tag="py3")
                    m3v = m3[:].rearrange("p (c f) -> p c f", c=32)
                    nc.vector.tensor_max(m3v[:], m2v[:, :, 0:32], m2v[:, :, 32:64])
                    sp = statp.tile([128, 32], f32, tag=f"sp{k}")
                    nc.vector.tensor_reduce(sp[:], m3v[:], axis=Ax.X, op=Alu.max)
                sp_parts[k] = sp

            def stats_finish():
                from concourse import bass_isa
                ctx = tc.high_priority()
                ctx.__enter__()
                # --- spatial sums (PE psum_sp, all 4 chunks) ---
                junkA = statp.tile([16, 256], f32)
                colA = statp.tile([16, 1], f32)
                nc.scalar.activation(junkA[:], psum_sp[:, 0:256], Act.Copy,
                                     accum_out=colA[:])
                junkB = statp.tile([16, 256], f32)
                colB = statp.tile([16, 1], f32)
                nc.scalar.activation(junkB[:], psum_sp[:, 256:512], Act.Copy,
                                     accum_out=colB[:])
                srow = statp.tile([1, 32], f32)
                nc.gpsimd.dma_start(srow[0:1, 0:32:2], colA[:])
                nc.gpsimd.dma_start(srow[0:1, 1:32:2], colB[:])
                nc.gpsimd.dma_start(cc_in[0:1, :], srow[:])
                # --- spatial max: combine 4 chunk partials ---
                mx01 = statp.tile([128, 32], f32)
                nc.vector.tensor_max(mx01[:], sp_parts[0][:], sp_parts[1][:])
                mx23 = statp.tile([128, 32], f32)
                nc.vector.tensor_max(mx23[:], sp_parts[2][:], sp_parts[3][:])
                mxa = statp.tile([128, 32], f32)
                nc.vector.tensor_max(mxa[:], mx01[:], mx23[:])
                mxr = statp.tile([128, 32], f32)
                nc.gpsimd.partition_all_reduce(mxr[:], mxa[:], 128,
                                               bass_isa.ReduceOp.max)
                nc.gpsimd.dma_start(cc_in[1:2, :], mxr[0:1, :])
                nc.gpsimd.collective_compute(
                    "AllGather", mybir.AluOpType.bypass,
                    replica_groups=PAIRS,
                    ins=[cc_in[:].opt()], outs=[cc_out[:].opt()])
                # gather the pair rows split by kind (sum rows 0,2; max
                # rows 1,3) so the combine stays entirely on the gpsimd queue
                gathS = statp.tile([2, 32], f32)
                nc.gpsimd.dma_start(gathS[:], cc_out[0:4:2, :])
                gathM = statp.tile([2, 32], f32)
                nc.gpsimd.dma_start(gathM[:], cc_out[1:4:2, :])
                ctx.__exit__(None, None, None)
                return gathS, gathM

            def ca_post(gaths):
                from concourse import bass_isa
                gathS, gathM = gaths
                ctx = tc.high_priority()
                ctx.__enter__()
                # pair-combine entirely on the gpsimd queue (no cross-engine
                # hops until fc1)
                tSa = statp.tile([2, 32], f32)
                nc.gpsimd.partition_all_reduce(tSa[:], gathS[:], 2,
                                               bass_isa.ReduceOp.add)
                tMa = statp.tile([2, 32], f32)
                nc.gpsimd.partition_all_reduce(tMa[:], gathM[:], 2,
                                               bass_isa.ReduceOp.max)
                hin = statp.tile([1, 64], f32)
                nc.gpsimd.dma_start(hin[0:1, 0:32], tSa[0:1, :])
                nc.gpsimd.dma_start(hin[0:1, 32:64], tMa[0:1, :])
                # fc1 via broadcast + fused mul-accumulate (all off the PE)
                hinb = statp.tile([128, 64], f32)
                nc.gpsimd.partition_broadcast(hinb[:], hin[:])
                junk1 = statp.tile([128, 64], f32)
                h1 = statp.tile([128, 1], f32)
                nc.vector.scalar_tensor_tensor(junk1[:], fc1w[:], 1.0, hinb[:],
                                               op0=Alu.bypass, op1=Alu.mult,
                                               accum_out=h1[:])
                hrelu = statp.tile([128, 1], f32)
                nc.vector.tensor_scalar(hrelu[:], h1[:], fc1b[:], 0.0,
                                        op0=Alu.add, op1=Alu.max)
                # fc2 on DVE too: ca0[c] = sum_j fc2w[c, j] * hrelu[j]
                hrow = statp.tile([1, 128], f32)
                nc.gpsimd.dma_start(hrow[:], hrelu[:])
                hrelB = statp.tile([32, 128], f32)
                nc.gpsimd.partition_broadcast(hrelB[:], hrow[:])
                junk2 = statp.tile([32, 128], f32)
                ca0 = statp.tile([32, 1], f32)
                nc.vector.scalar_tensor_tensor(junk2[:], fc2w[:], 1.0, hrelB[:],
                                               op0=Alu.bypass, op1=Alu.mult,
                                               accum_out=ca0[:])
                ca_col = statp.tile([32, 1], f32)
                nc.scalar.activation(ca_col[:], ca0[:], Act.Sigmoid, bias=fc2b[:])
                ca_row = statp.tile([1, 32], f32)
                nc.gpsimd.dma_start(ca_row[:], ca_col[:])
                nc.gpsimd.partition_broadcast(ca_rep[:], ca_row[:])
                ctx.__exit__(None, None, None)

            # fp8 DoubleRow conv: tap pairs (2j, 2j+1) share one matmul.
            # convw layout: [p, j(24), ph(2), two(2), col(128)] + tail [p, ph(2), col(128)]
            cwv = convw[:, :NPAIR * 512].rearrange("p (j ph two c) -> p j ph two c",
                                                   j=NPAIR, ph=2, two=2)
            cwtail = convw[:, NPAIR * 512:].rearrange("p (t c) -> p t c", c=128)
            sflat = s_conv[:]

            def conv_rhs(g, j):
                t0, t1 = TAP_PAIRS[j]
                kz, ky = t0 // 7, t0 % 7
                delta = (t1 // 7 - kz) * HP + (t1 % 7 - ky)
                off = (8 * g + 1 + kz) * HP + ky
                return bass.AP(tensor=sflat.tensor,
                               offset=sflat.offset + off,
                               ap=[list(sflat.ap[0]), [delta, 2], [HP, 8], [1, 64]])

            def conv_group(g):
                # outputs own planes d_own in [8g, 8g+8) = local d in [8g+4, 8g+12)
                pc_a = pconvp.tile([128, 512], f32, tag="pconv")
                pc_b = pconvp.tile([128, 512], f32, tag="pconv")
                pc = [pc_a, pc_b]
                for j in range(NPAIR):
                    rhs = conv_rhs(g, j)
                    for ph in range(2):
                        nc.tensor.matmul(pc[ph][:], cwv[:, j, ph], rhs,
                                         start=(j == 0), stop=False,
                                         perf_mode=DR, skip_group_check=True)
                # tail tap (kz=6, ky=5), plain fp8 matmul
                toff = (8 * g + 7) * HP + 5
                trhs = bass.AP(tensor=sflat.tensor, offset=sflat.offset + toff,
                               ap=[list(sflat.ap[0]), [HP, 8], [1, 64]])
                for ph in range(2):
                    nc.tensor.matmul(pc[ph][:], cwtail[:, ph], trhs,
                                     start=False, stop=True, skip_group_check=True)
                # relu -> sbuf (descale the fp8 weight pre-scale)
                for ph in range(2):
                    r = relup.tile([128, 512], bf16, tag="relu")
                    nc.scalar.activation(r[:], pc[ph][:], Act.Relu, scale=1.0 / WS)
                    relu_tiles[g][ph] = r
                # conv2 (1x1x1, 4 -> 1) and sigmoid
                psa = pmiscp.tile([64, 512], f32, tag="m")
                nc.tensor.matmul(psa[:], c2w[:, 0:64], relu_tiles[g][0][:],
                                 start=True, stop=False, skip_group_check=True)
                nc.tensor.matmul(psa[:], c2w[:, 64:128], relu_tiles[g][1][:],
                                 start=False, stop=True, skip_group_check=True)
                sa_w = sawp.tile([64, 512], f32, tag="saw")
                nc.scalar.activation(sa_w[:], psa[:], Act.Copy)
                # transpose [64,128] blocks -> sa128, sigmoid fused in the copy
                for b4 in range(4):
                    pt = ptpp.tile([128, 64], f32, tag="ptp")
                    nc.tensor.transpose(pt[:], sa_w[:, b4 * 128:(b4 + 1) * 128],
                                        ident[:])
                    col = (4 * g + b4) * 64
                    nc.scalar.activation(sa128[:, col:col + 64], pt[:], Act.Sigmoid)

            def output_quarter(g):
                # outputs for d_own in [8g, 8g+8): sa128 cols [g*256, (g+1)*256)
                # big [128, 4096] tiles (4 channel-groups) -> 1 DMA + 1 fat
                # anti op per half; attn DMA on sync, anti DMA on scalar
                sl_sa = slice(g * 256, (g + 1) * 256)
                for half in range(2):
                    abuf = outp.tile([128, 4096], bf16, tag="abuf")
                    bbuf = outp.tile([128, 4096], bf16, tag="bbuf")
                    for c16 in range(16):
                        c = half * 16 + c16
                        nc.vector.tensor_scalar_mul(
                            abuf[:, c16 * 256:(c16 + 1) * 256], sa128[:, sl_sa],
                            ca_rep[:, c:c + 1])
                    nc.vector.tensor_scalar(bbuf[:], abuf[:], -1.0, 1.0,
                                            op0=Alu.mult, op1=Alu.add)
                    nc.sync.dma_start(attn_ext[g, half], abuf[:])
                    nc.scalar.dma_start(anti_ext[g, half], bbuf[:])

            # ---- schedule ----
            # PE order: stats matmuls of chunks {0,1,4} -> conv g0 ->
            # chunk 2 -> chunk 3 (all ca-critical stats before conv g1) ->
            # conv g1..g3.  DVE order: spatial-max pyramid before the
            # channel-max tree per chunk; chunk-3 tree after the collective
            # is issued.
            xdma_all()
            for k in (0, 1):
                psh = chsum(k, do_spsum=True)
                spmax(k)
                chmax(k, psh)
            psh_2 = chsum(2, do_spsum=True)
            spmax(2)
            chmax(2, psh_2)
            psh_4 = chsum(4)
            chmax(4, psh_4)
            psh_3 = chsum(3, do_spsum=True)
            spmax(3)
            gath = stats_finish()
            conv_group(0)
            chmax(3, psh_3)
            ca_post(gath)
            conv_group(1)
            conv_group(2)
            conv_group(3)
            output_quarter(0)
            output_quarter(1)
            output_quarter(2)
            output_quarter(3)

    nc.compile()
    return nc


def _host_inputs(x, fc1_w, fc1_b, fc2_w, fc2_b, conv1_w, conv2_w):
    """Build the per-core input maps (all host-side numpy)."""
    x = np.asarray(x, dtype=np.float32)
    # conv1 Toeplitz lhsT blocks: T[t2][(i,w_in), (o2,w_out)]
    w1 = np.asarray(conv1_w, dtype=np.float32)  # [4, 2, 7, 7, 7]
    T = np.zeros((98, 128, 128), np.float32)
    for kz in range(7):
        for ky in range(7):
            t = kz * 7 + ky
            for pair in range(2):
                t2 = t * 2 + pair
                for o2 in range(2):
                    oc = pair * 2 + o2
                    for i in range(2):
                        for dk in range(7):
                            off = dk - 3  # w_in = w_out + off
                            wv = w1[oc, i, kz, ky, dk]
                            if off >= 0:
                                wo = np.arange(0, 64 - off)
                            else:
                                wo = np.arange(-off, 64)
                            T[t2, i * 64 + wo + off, o2 * 64 + wo] = wv
    T *= WS
    # pack DoubleRow pairs: [row, j, ph, two, col]; tail taps 48 at the end
    cw8 = np.zeros((128, NPAIR * 2 * 2 * 128 + 2 * 128), np.float32)
    cwv = cw8[:, :NPAIR * 2 * 2 * 128].reshape(128, NPAIR, 2, 2, 128)
    for j in range(NPAIR):
        for ph in range(2):
            for two in range(2):
                cwv[:, j, ph, two, :] = T[TAP_PAIRS[j][two] * 2 + ph]
    for ph in range(2):
        cw8[:, NPAIR * 512 + ph * 128:NPAIR * 512 + (ph + 1) * 128] = T[TAP_SINGLE * 2 + ph]
    convw8 = cw8.astype(F8)

    # pair one-hot weights for the PE spatial-sum matmuls: matmul m covers
    # channels (2m, 2m+1); psum row m gets their partition sums by f-half
    ohpb = np.zeros((128, 256), F16)
    for m in range(16):
        ohpb[:, m * 16 + m] = 1.0
    ident = np.eye(64, dtype=np.float32)
    # two identity blocks for the DoubleRow channel-sum + a plain bf16 one
    identb = np.concatenate([np.eye(128), np.eye(128)], axis=1).astype(F8)
    identbB = np.eye(128, dtype=np.float32).astype(F16)
    swapA = np.zeros((128, 64), np.float32)
    swapA[64:128] = np.eye(64)
    swapA = swapA.astype(F8)
    swapB = np.eye(64, dtype=np.float32).astype(F16)

    c2v = np.asarray(conv2_w, dtype=np.float32).reshape(4)
    c2 = np.zeros((128, 128), np.float32)
    for pair in range(2):
        for o2 in range(2):
            w = np.arange(64)
            c2[o2 * 64 + w, pair * 64 + w] = c2v[pair * 2 + o2]
    c2 = c2.astype(F16)

    fc1_w = np.asarray(fc1_w, np.float32)           # [128, 64]
    fc1s = fc1_w.copy()
    fc1s[:, 0:32] *= 1.0 / NVOX
    fc1bv = np.asarray(fc1_b, np.float32).reshape(128, 1)
    fc2v = np.ascontiguousarray(np.asarray(fc2_w, np.float32))  # [32, 128]
    masks = np.zeros((4, 2), np.float32)
    masks[0, 0] = masks[2, 0] = 1.0
    masks[1, 1] = masks[3, 1] = 1.0
    fc2bv = np.asarray(fc2_b, np.float32).reshape(32, 1)

    in_maps = []
    for r in range(NCORES):
        b, dhalf = r // 2, r % 2
        xp = np.zeros((C, DL, H, W), np.float32)
        if dhalf == 0:
            xp[:, 4:40] = x[b, :, 0:36]
        else:
            xp[:, 0:36] = x[b, :, 28:64]
        # chunk remap: chunks 0-3 carry own planes 4..35, chunk 4 the halos
        xp = xp[:, list(range(4, 36)) + list(range(0, 4)) + list(range(36, 40))]
        # [c, k, dl, hh, h2, w] -> [k, h2, w, c, dl, hh] -> [5, 128, 8192]
        xr = xp.reshape(C, NCHUNK, CP, 32, 2, W).transpose(1, 4, 5, 0, 2, 3)
        xhost = np.ascontiguousarray(xr.reshape(NCHUNK, 128, 32 * PFC)).astype(F16)

        in_maps.append({
            "x": xhost, "convw": convw8, "ohpb": ohpb, "ident": ident, "identb": identb, "identbB": identbB, "c2w": c2,
            "fc1w": fc1s, "fc1b": fc1bv, "fc2w": fc2v, "fc2b": fc2bv,
            "masks": masks, "swapA": swapA, "swapB": swapB,
        })
    return in_maps


def _decode_out(arr):
    """[4, 2, 128, 4096] -> [C, 32, H, W] (own planes)."""
    a = np.asarray(arr, dtype=np.float32)
    a = a.reshape(4, 2, 2, 64, 16, 4, 64)           # g, half, d2, h, c16, b4, w
    a = a.transpose(1, 4, 0, 5, 2, 3, 6)            # half, c16, g, b4, d2, h, w
    return a.reshape(C, 32, H, W)


def _install_ntff_shim():
    """The agent image's antenv lacks axon_hooks; recreate it so
    run_bass_kernel_spmd(trace=True) can NTFF-profile via libaxon."""
    import sys, types, contextlib, ctypes
    try:
        import antenv.axon_hooks  # noqa
        return
    except ImportError:
        pass
    so_path = "/opt/axon/libaxon_pjrt.so"
    lib = ctypes.CDLL(so_path)
    if not hasattr(lib, "axon_start_nrt_profile"):
        return
    lib.axon_start_nrt_profile.argtypes = [ctypes.POINTER(ctypes.c_int64),
                                           ctypes.c_size_t]
    lib.axon_start_nrt_profile.restype = ctypes.c_int64
    lib.axon_stop_nrt_profile.argtypes = [ctypes.c_char_p]
    lib.axon_stop_nrt_profile.restype = ctypes.c_int64

    @contextlib.contextmanager
    def _hook(output_dir, device_ids):
        import jax
        jax.devices()
        if device_ids:
            ids = (ctypes.c_int64 * len(device_ids))(*device_ids)
            rc = lib.axon_start_nrt_profile(ids, len(device_ids))
        else:
            rc = lib.axon_start_nrt_profile(None, 0)
        if rc != 0:
            raise RuntimeError(f"axon_start_nrt_profile rc={rc}")
        try:
            yield
        finally:
            n = lib.axon_stop_nrt_profile(str(output_dir).encode())
            print(f"profile: {n} file(s) written to {output_dir}")

    mod = types.ModuleType("antenv.axon_hooks")
    _state = {"hook": _hook}
    mod.get_axon_ntff_profile_hook = lambda: _state["hook"]
    mod.set_axon_ntff_profile_hook = lambda h: _state.__setitem__("hook", h)
    sys.modules["antenv.axon_hooks"] = mod


def kernel(x, fc1_w, fc1_b, fc2_w, fc2_b, conv1_w, conv2_w, _want_time=False):
    from concourse.bass_utils import run_bass_kernel_spmd
    if _want_time:
        _install_ntff_shim()

    if "nc" not in _CACHE:
        _CACHE["nc"] = _build_nc()
    nc = _CACHE["nc"]

    in_maps = _host_inputs(x, fc1_w, fc1_b, fc2_w, fc2_b, conv1_w, conv2_w)
    res = run_bass_kernel_spmd(nc, in_maps, core_ids=list(range(NCORES)),
                               trace=bool(_want_time))
    attention = np.empty((B, C, D, H, W), np.float32)
    anti = np.empty((B, C, D, H, W), np.float32)
    for r in range(NCORES):
        b, dhalf = r // 2, r % 2
        d0 = dhalf * 32
        attention[b, :, d0:d0 + 32] = _decode_out(res.results[r]["attn"])
        anti[b, :, d0:d0 + 32] = _decode_out(res.results[r]["anti"])
    if _want_time:
        return (attention, anti), res.exec_time_ns
    return attention, anti
